# revision 1
# baseline (speedup 1.0000x reference)
"""Dilated segment attention on 8 TRN2 NeuronCores (Bass/Tile).

Problem (hardcoded from spec):
  x [2, 8192, 2048] f32, Wqkv [6144, 2048], b_qkv [6144], Wout [2048, 2048],
  b_out [2048].  segment=512, dilation=2 -> 16 segments of L=256 dilated
  tokens per batch; per-segment 16-head attention (hd=128); fused qkv and
  out projections.  Output [2, 4096, 2048] f32.

Sharding: the 32 (batch, segment) instances are independent -> 4 per core.
Host pre-gathers the dilated tokens, pre-transposes/pre-tiles operands and
casts to bf16 (compute precision; measured end-to-end rel err ~5e-3).

Per-core dataflow (all matmuls K=128, bf16):
  qkv proj   : feature-major  qkvT[e, tok] = W-tile.T @ xsT-tile  (accum 16 d-tiles)
  scores     : scores[lq, lk] = qT.T @ kT          (per seg, head)
  softmax    : exp on ScalarE (scale=1/sqrt(hd), accum_out row sums; scores
               are provably in [-6, 6] so no max subtraction), normalize on DVE
  attn.T     : PE transpose (128x128 tiles)
  AV         : outT[hd, lq] = v[lk, hd].T @ attnT[lk, lq]
  out proj   : out[l, e] = aT-tile.T @ WoutT-tile  (accum 16 head-tiles,
               token-major, so the HBM store is linear)
b_out is applied on the host (purely linear post-op); b_qkv is applied
on-chip (ScalarE bias) since it feeds the softmax nonlinearity.
"""

import numpy as np
import ml_dtypes

B = 2
S = 8192
D = 2048
H = 16
HD = 128
SEGMENT = 512
DIL = 2
NSEG = S // SEGMENT          # 16
L = SEGMENT // DIL           # 256 dilated tokens per segment
N_CORES = 8
PAIRS = B * NSEG             # 32 independent (b, n) instances
SPC = PAIRS // N_CORES       # 4 segments per core
TOK = SPC * L                # 1024 tokens per core
DT = D // 128                # 16 contraction tiles
NCHUNK = 3 * D // 128        # 48 qkv feature chunks (16 q, 16 k, 16 v)
SCALE = 1.0 / float(np.sqrt(HD))

_PROGRAM = None


def _build_program():
    import concourse.bass as bass
    import concourse.bacc as bacc
    import concourse.tile as tile
    from concourse import mybir

    BF = mybir.dt.bfloat16
    F32 = mybir.dt.float32
    ts = bass.ts

    nc = bacc.Bacc("TRN2", target_bir_lowering=False, debug=False,
                   num_devices=N_CORES)

    xst_d = nc.dram_tensor("xst", [128, DT * TOK], BF, kind="ExternalInput")
    wqkv_d = nc.dram_tensor("wqkv_t", [NCHUNK, 128, DT * 128], BF,
                            kind="ExternalInput")
    wout_d = nc.dram_tensor("wout_t", [4, 128, DT * 512], BF, kind="ExternalInput")
    bq_d = nc.dram_tensor("bq_t", [128, NCHUNK], F32, kind="ExternalInput")
    out_d = nc.dram_tensor("out", [TOK, D], F32, kind="ExternalOutput")

    with tile.TileContext(nc) as tc:
        with (
            tc.tile_pool(name="const", bufs=1) as const_p,
            tc.tile_pool(name="big", bufs=1) as big_p,
            tc.tile_pool(name="wq", bufs=6) as w_p,
            tc.tile_pool(name="qk", bufs=4) as qk_p,
            tc.tile_pool(name="vt", bufs=2) as vt_p,
            tc.tile_pool(name="ex", bufs=4) as ex_p,
            tc.tile_pool(name="st", bufs=2) as st_p,
            tc.tile_pool(name="ou", bufs=2) as ou_p,
            tc.tile_pool(name="pp", bufs=4, space="PSUM") as pp_p,
            tc.tile_pool(name="pa", bufs=2, space="PSUM") as pa_p,
        ):
            bq_sb = const_p.tile([128, NCHUNK], F32)
            nc.sync.dma_start(out=bq_sb[:], in_=bq_d[:])
            ones = const_p.tile([128, 1], BF)
            nc.gpsimd.memset(ones[:], 1.0)

            # One linear 512KB DMA per weight chunk and 4 x 1MB for xst:
            # the Sync sequencer dispatches each DMA in ~600ns, so hundreds
            # of small tile DMAs would serialize into multi-us delivery
            # latency at every chunk boundary.
            first_w = w_p.tile([128, DT * 128], BF, tag="w", name="first_w")
            nc.sync.dma_start(out=first_w[:], in_=wqkv_d[32])
            xst_sb = big_p.tile([128, DT, TOK], BF)
            for kk in range(4):
                nc.sync.dma_start(
                    out=xst_sb[:, 4 * kk:4 * (kk + 1), :],
                    in_=xst_d[:, 4 * kk * TOK:4 * (kk + 1) * TOK],
                )
            vtok_sb = big_p.tile([128, H, SPC * 2, 128], BF)
            aT_sb = big_p.tile([128, SPC, H, L], BF)

            def proj_chunk(c, out_tile, wck=None):
                """qkvT chunk c: out_tile[128, TOK] bf16 = (Wqkv chunk).T @ xsT + b."""
                if wck is None:
                    wck = w_p.tile([128, DT * 128], BF, tag="w")
                    nc.sync.dma_start(out=wck[:], in_=wqkv_d[c])
                pss = [pp_p.tile([128, 512], F32, tag="pp", name=f"ps{half}")
                       for half in range(2)]
                for dt in range(DT):
                    for half in range(2):
                        nc.tensor.matmul(
                            pss[half][:],
                            wck[:, ts(dt, 128)],
                            xst_sb[:, dt, ts(half, 512)],
                            start=(dt == 0),
                            stop=(dt == DT - 1),
                        )
                for half in range(2):
                    nc.scalar.activation(
                        out=out_tile[:, ts(half, 512)],
                        in_=pss[half][:],
                        func=mybir.ActivationFunctionType.Identity,
                        bias=bq_sb[:, c:c + 1],
                        scale=1.0,
                    )

            # ---- v projection (feature-major) + transpose to token-major ----
            # One transposing DMA per head (xbar transpose, ~261GB/s): row
            # tok = tc*128+p of vt.T lands at vtok[p, tc, :], exactly the AV
            # stationary layout.  Keeps ~28us of transposes off the PE.
            # Emitted one chunk behind the projection so the DMA never waits
            # on the ScalarE psum->sbuf drain in the static stream.
            def v_transposes(h, vt_tile):
                nc.sync.dma_start(out=vtok_sb[:, h, :, :], in_=vt_tile[:],
                                  transpose=True)

            prev_v = None
            for h in range(H):
                vt_tile = vt_p.tile([128, TOK], BF, tag="vt")
                proj_chunk(32 + h, vt_tile, wck=first_w if h == 0 else None)
                if prev_v is not None:
                    v_transposes(h - 1, prev_v)
                prev_v = vt_tile
            v_transposes(H - 1, prev_v)

            # ---- per-head: q/k projection then attention over 4 segments ----
            # Wout tile DMAs are interleaved one-per-head so the 8.4MB burst
            # never starves the per-head W-tile loads (a single burst at the
            # v->attention boundary measured a 20us PE stall + HAM rethrottle).
            # Attention for head h is emitted after head h+1's projection:
            # the next head's ~26us of projection matmuls hide the softmax
            # ACT->PE->DVE->GpSimd->DVE chain in the static PE stream.

            def attention_head(h, qh, kh):
                # scoresT[lk, lq] directly (operands swapped): exp is
                # layout-agnostic (scores provably small -> no max pass),
                # softmax sums go along partitions via a ones-matmul, expT
                # feeds AV untransposed, and the normalization happens at
                # the psum->sbuf copy of the AV output, so the reciprocal /
                # partition-broadcast chain never blocks the PE stream.
                # scT of seg+1 is emitted before sums/AV of seg so the exp
                # ACT latency hides behind PE work.
                scts = [None] * SPC

                def emit_scT(seg):
                    scT = pa_p.tile([128, 2, L], F32, tag="pa", name="scT")
                    for lkc in range(2):
                        nc.tensor.matmul(
                            scT[:, lkc, :],
                            kh[:, seg * L + lkc * 128: seg * L + (lkc + 1) * 128],
                            qh[:, seg * L:(seg + 1) * L],
                        )
                    scts[seg] = scT

                emit_scT(0)
                for seg in range(SPC):
                    if seg + 1 < SPC:
                        emit_scT(seg + 1)
                    e_t = ex_p.tile([128, 2, L], BF, tag="ex")
                    nc.scalar.activation(
                        out=e_t[:],
                        in_=scts[seg][:],
                        func=mybir.ActivationFunctionType.Exp,
                        scale=SCALE,
                    )
                    scts[seg] = None
                    # av ([:, 0, :]) and the softmax sums row ([0:1, 1, :])
                    # share one PSUM bank; Tile serializes the cross-use.
                    avs = pa_p.tile([128, 2, L], F32, tag="pav", bufs=2,
                                    name="avs")
                    for lkc in range(2):
                        nc.tensor.matmul(
                            avs[0:1, 1, :],
                            ones[:],
                            e_t[:, lkc, :],
                            start=(lkc == 0),
                            stop=(lkc == 1),
                        )
                    for lkc in range(2):
                        nc.tensor.matmul(
                            avs[:, 0, :],
                            vtok_sb[:, h, seg * 2 + lkc, :],
                            e_t[:, lkc, :],
                            start=(lkc == 0),
                            stop=(lkc == 1),
                        )
                    inv = st_p.tile([1, L], F32, tag="st")
                    nc.vector.reciprocal_approx_fast(out=inv[:], in_=avs[0:1, 1, :])
                    invB = ex_p.tile([128, L], F32, tag="invb")
                    nc.gpsimd.partition_broadcast(invB[:], inv[:])
                    nc.vector.tensor_mul(aT_sb[:, seg, h, :], avs[:, 0, :], invB[:])

            prev_qk = None
            for h in range(H):
                qh = qk_p.tile([128, TOK], BF, tag="qk")
                kh = qk_p.tile([128, TOK], BF, tag="qk")
                proj_chunk(h, qh)
                proj_chunk(16 + h, kh)
                if prev_qk is not None:
                    attention_head(h - 1, *prev_qk)
                prev_qk = (qh, kh)
            attention_head(H - 1, *prev_qk)

            # ---- output projection (token-major) ----
            # Wout is streamed in four 2MB e-quarters (one linear DMA each)
            # instead of held resident; the freed 32KB/partition goes to
            # deeper W-chunk prefetch.  LDWEIGHTS (one per aT tile per
            # quarter) hides under the previous matmul's streaming.
            for eq in range(4):
                wq_t = w_p.tile([128, DT, 512], BF, tag="wo", bufs=2,
                                name="wq_t")
                nc.sync.dma_start(out=wq_t[:], in_=wout_d[eq])
                for lc in range(TOK // 128):
                    seg, lqc = lc // 2, lc % 2
                    po = pp_p.tile([128, 512], F32, tag="pp", name="po")
                    for dt in range(DT):
                        nc.tensor.matmul(
                            po[:],
                            aT_sb[:, seg, dt, ts(lqc, 128)],
                            wq_t[:, dt, :],
                            start=(dt == 0),
                            stop=(dt == DT - 1),
                        )
                    ob = ou_p.tile([128, 512], F32, tag="ou")
                    nc.vector.tensor_copy(out=ob[:], in_=po[:])
                    nc.sync.dma_start(
                        out=out_d[lc * 128:(lc + 1) * 128,
                                  eq * 512:(eq + 1) * 512],
                        in_=ob[:],
                    )

    nc.compile()
    _dedupe_ldweights(nc)
    return nc


def _dedupe_ldweights(nc):
    """Drop InstLdweights whose weights are already resident in the PE array.

    tile_legalize emits one LDWEIGHTS per matmul; consecutive matmuls that
    share the stationary operand (projection token-halves, out-proj eq
    pairs) reload identical weights, costing ~97ns of PE pipe each.  Walk
    each block's PE stream tracking the loaded-weights key and delete
    reloads.  Only semaphore-free LDWEIGHTS are dropped, so the sync graph
    is untouched; EVENT_SEMAPHORE/DRAIN between pairs don't disturb the
    array, any other PE instruction conservatively invalidates the key.
    """
    from concourse import mybir

    PE = mybir.EngineType.PE
    dropped = 0
    for f in nc.m.functions:
        for blk in f.blocks:
            insts = blk.instructions
            loaded = None
            to_drop = []
            for idx, x in enumerate(insts):
                if getattr(x, "engine", None) != PE:
                    continue
                nm = type(x).__name__
                if nm == "InstLdweights":
                    si = x.sync_info
                    clean = si is None or (not si.on_wait and not si.on_update)
                    key = (str(x.ins[0]), str(x.is_transpose),
                           str(x.perf_mode), str(x.tile_position))
                    if clean and loaded == key:
                        to_drop.append(idx)
                    else:
                        loaded = key
                elif nm == "InstMatmult":
                    continue
                elif nm in ("InstEventSemaphore", "InstDrain"):
                    continue
                else:
                    loaded = None
            for idx in reversed(to_drop):
                del insts[idx]
            blk.instructions = insts
            dropped += len(to_drop)
    return dropped


def get_program():
    global _PROGRAM
    if _PROGRAM is None:
        _PROGRAM = _build_program()
    return _PROGRAM


def make_in_maps(x, Wqkv, b_qkv):
    """Host-side shard + layout prep (bf16 casts, transposes, tiling)."""
    bf16 = ml_dtypes.bfloat16
    x = np.asarray(x, dtype=np.float32)
    Wqkv = np.asarray(Wqkv, dtype=np.float32)
    b_qkv = np.asarray(b_qkv, dtype=np.float32)

    xs = x.reshape(B, NSEG, SEGMENT, D)[:, :, ::DIL, :]     # [2,16,256,2048]
    xs_flat = xs.reshape(PAIRS, L, D)

    # lhsT tiles packed partition-major: wt[c, p, dt*128+j] = WqkvT[dt*128+p,
    # c*128+j] so one chunk is a single linear per-partition DMA.
    wt = np.ascontiguousarray(
        Wqkv.reshape(NCHUNK, 128, DT, 128).transpose(0, 3, 2, 1)
        .reshape(NCHUNK, 128, DT * 128)
    ).astype(bf16)                                          # [48,128,2048]
    bqt = np.ascontiguousarray(b_qkv.reshape(NCHUNK, 128).T)  # [128,48] f32

    in_maps = []
    for i in range(N_CORES):
        tok = xs_flat[SPC * i:SPC * (i + 1)].reshape(TOK, D)
        xst = np.ascontiguousarray(
            tok.T.reshape(DT, 128, TOK).transpose(1, 0, 2)
            .reshape(128, DT * TOK)).astype(bf16)
        in_maps.append({"xst": xst, "wqkv_t": wt, "bq_t": bqt})
    return in_maps


def make_wout_tiled(Wout):
    Wout = np.asarray(Wout, dtype=np.float32)
    # [eq, p, dt*512+j] = Wout[eq*512+j, dt*128+p]: one linear DMA/quarter
    return np.ascontiguousarray(
        Wout.T.reshape(DT, 128, 4, 512).transpose(2, 1, 0, 3)
        .reshape(4, 128, DT * 512)).astype(ml_dtypes.bfloat16)


def kernel(x, Wqkv, b_qkv, Wout, b_out):
    from concourse import bass_utils

    nc = get_program()
    in_maps = make_in_maps(x, Wqkv, b_qkv)
    wot = make_wout_tiled(Wout)
    for m in in_maps:
        m["wout_t"] = wot

    res = bass_utils.run_bass_kernel_spmd(
        nc, in_maps, core_ids=list(range(N_CORES)))
    outs = [res.results[i]["out"] for i in range(N_CORES)]
    full = np.concatenate(outs, axis=0) + np.asarray(b_out, dtype=np.float32)
    return np.ascontiguousarray(full.reshape(B, NSEG * L, D), dtype=np.float32)



# revision 4
# speedup vs baseline: 1.0348x; 1.0348x over previous
"""Dilated segment attention on 8 TRN2 NeuronCores (Bass/Tile).

Problem (hardcoded from spec):
  x [2, 8192, 2048] f32, Wqkv [6144, 2048], b_qkv [6144], Wout [2048, 2048],
  b_out [2048].  segment=512, dilation=2 -> 16 segments of L=256 dilated
  tokens per segment per batch; per-segment 16-head attention (hd=128);
  fused qkv and out projections.  Output [2, 4096, 2048] f32.

Sharding: the 32 (batch, segment) instances are independent -> 4 per core.
Host pre-gathers the dilated tokens, pre-transposes/pre-tiles operands and
casts to bf16 (compute precision; measured end-to-end rel err ~5e-3).

The kernel is PE-bound (93%+ occupancy): 1.11M matmul columns/core at
2.4GHz is ~464us.  Beyond the baseline, this version
  - computes softmax denominators with a per-head batched GpSimd
    partition_all_reduce instead of ones-matmuls (-32k PE columns),
  - interleaves attention seg-pairs between the q- and k-projection
    chunks so the ACT exp latency always hides under ~7us of projection,
  - emits the output projection feature-major (outT[e, tok]) so the
    stationary operand is a Wout tile reused for 1024 moving columns
    (half the LDWEIGHTS of the token-major form); host transposes back,
  - streams xst by token-quarters and runs the first v-chunk
    quarter-major so the PE starts ~4us earlier during the cold 4MB
    xst delivery.

Per-core dataflow (all matmuls K=128, bf16):
  qkv proj   : feature-major  qkvT[e, tok] = W-tile.T @ xsT-tile  (accum 16)
  scores     : scoresT[lk, lq] = kT.T @ qT  (per seg, head)
  softmax    : exp on ScalarE (scale=1/sqrt(hd); scores provably in
               [-6, 6] so no max subtraction); denominators via DVE
               chunk-add + GpSimd partition_all_reduce; normalize on DVE
               at the AV psum drain.
  AV         : outT[hd, lq] = v[lk, hd].T @ expT[lk, lq]
  out proj   : outT[e, tok] = WoutT-tile.T @ aT-tile  (accum 16 heads)
b_out is applied on the host (purely linear post-op); b_qkv is applied
on-chip (ScalarE bias) since it feeds the softmax nonlinearity.
"""

import numpy as np
import ml_dtypes

B = 2
S = 8192
D = 2048
H = 16
HD = 128
SEGMENT = 512
DIL = 2
NSEG = S // SEGMENT          # 16
L = SEGMENT // DIL           # 256 dilated tokens per segment
N_CORES = 8
PAIRS = B * NSEG             # 32 independent (b, n) instances
SPC = PAIRS // N_CORES       # 4 segments per core
TOK = SPC * L                # 1024 tokens per core
DT = D // 128                # 16 contraction tiles
NCHUNK = 3 * D // 128        # 48 qkv feature chunks (16 q, 16 k, 16 v)
SCALE = 1.0 / float(np.sqrt(HD))

_PROGRAM = None


def _build_program():
    import concourse.bass as bass
    import concourse.bacc as bacc
    import concourse.tile as tile
    from concourse import mybir
    from concourse import bass_isa

    BF = mybir.dt.bfloat16
    F32 = mybir.dt.float32
    ts = bass.ts

    nc = bacc.Bacc("TRN2", target_bir_lowering=False, debug=False,
                   num_devices=N_CORES)

    xst_d = nc.dram_tensor("xst", [128, DT, TOK], BF, kind="ExternalInput")
    wqkv_d = nc.dram_tensor("wqkv_t", [NCHUNK, 128, DT * 128], BF,
                            kind="ExternalInput")
    wout_d = nc.dram_tensor("wout_t", [DT, 128, H * 128], BF,
                            kind="ExternalInput")
    bq_d = nc.dram_tensor("bq_t", [128, NCHUNK], F32, kind="ExternalInput")
    out_d = nc.dram_tensor("out", [D, TOK], F32, kind="ExternalOutput")

    with tile.TileContext(nc) as tc:
        with (
            tc.tile_pool(name="const", bufs=1) as const_p,
            tc.tile_pool(name="big", bufs=1) as big_p,
            tc.tile_pool(name="wq", bufs=7) as w_p,
            tc.tile_pool(name="qk", bufs=4) as qk_p,
            tc.tile_pool(name="vt", bufs=2) as vt_p,
            tc.tile_pool(name="ex", bufs=4) as ex_p,
            tc.tile_pool(name="st", bufs=2) as st_p,
            tc.tile_pool(name="ou", bufs=2) as ou_p,
            tc.tile_pool(name="pp", bufs=4, space="PSUM") as pp_p,
            tc.tile_pool(name="pa", bufs=2, space="PSUM") as pa_p,
        ):
            bq_sb = const_p.tile([128, NCHUNK], F32)
            nc.sync.dma_start(out=bq_sb[:], in_=bq_d[:])

            # Cold-start ordering: the PE's first work (v chunk 0, index
            # 32) needs its W chunk and the first token-quarter of xst.
            # One linear 512KB DMA for the W chunk, then xst in 4
            # token-quarter DMAs (strided, 512B runs) so quarter 0 lands
            # ~3x sooner than the full 4MB.
            first_w = w_p.tile([128, DT * 128], BF, tag="w", name="first_w")
            nc.sync.dma_start(out=first_w[:], in_=wqkv_d[32])
            xst_sb = big_p.tile([128, DT, TOK], BF)
            for q in range(4):
                nc.sync.dma_start(
                    out=xst_sb[:, :, ts(q, 256)],
                    in_=xst_d[:, :, ts(q, 256)],
                )
            vtok_sb = big_p.tile([128, H, SPC * 2, 128], BF)
            aT_sb = big_p.tile([128, SPC, H, L], BF)

            def proj_chunk(c, out_tile, wck=None, quarter_major=False):
                """qkvT chunk c: out_tile[128, TOK] bf16 = W-chunk.T @ xsT + b."""
                if wck is None:
                    wck = w_p.tile([128, DT * 128], BF, tag="w")
                    nc.sync.dma_start(out=wck[:], in_=wqkv_d[c])
                if quarter_major:
                    # first chunk while xst streams in: consume one
                    # token-quarter at a time so matmuls start on quarter 0
                    for q in range(4):
                        psq = pp_p.tile([128, 512], F32, tag="pp", name="psq")
                        for dt in range(DT):
                            nc.tensor.matmul(
                                psq[:, 0:256],
                                wck[:, ts(dt, 128)],
                                xst_sb[:, dt, ts(q, 256)],
                                start=(dt == 0),
                                stop=(dt == DT - 1),
                            )
                        nc.scalar.activation(
                            out=out_tile[:, ts(q, 256)],
                            in_=psq[:, 0:256],
                            func=mybir.ActivationFunctionType.Identity,
                            bias=bq_sb[:, c:c + 1],
                            scale=1.0,
                        )
                    return
                pss = [pp_p.tile([128, 512], F32, tag="pp", name=f"ps{half}")
                       for half in range(2)]
                for dt in range(DT):
                    for half in range(2):
                        nc.tensor.matmul(
                            pss[half][:],
                            wck[:, ts(dt, 128)],
                            xst_sb[:, dt, ts(half, 512)],
                            start=(dt == 0),
                            stop=(dt == DT - 1),
                        )
                for half in range(2):
                    nc.scalar.activation(
                        out=out_tile[:, ts(half, 512)],
                        in_=pss[half][:],
                        func=mybir.ActivationFunctionType.Identity,
                        bias=bq_sb[:, c:c + 1],
                        scale=1.0,
                    )

            # ---- v projection (feature-major) + transpose to token-major ----
            # One transposing DMA per head (xbar transpose, ~261GB/s): row
            # tok = tc*128+p of vt.T lands at vtok[p, tc, :], exactly the AV
            # stationary layout.  Emitted one chunk behind the projection so
            # the DMA never waits on the ScalarE psum->sbuf drain.
            def v_transposes(h, vt_tile):
                nc.sync.dma_start(out=vtok_sb[:, h, :, :], in_=vt_tile[:],
                                  transpose=True)

            prev_v = None
            for h in range(H):
                vt_tile = vt_p.tile([128, TOK], BF, tag="vt")
                proj_chunk(32 + h, vt_tile,
                           wck=first_w if h == 0 else None,
                           quarter_major=(h == 0))
                if prev_v is not None:
                    v_transposes(h - 1, prev_v)
                prev_v = vt_tile
            v_transposes(H - 1, prev_v)

            # ---- per-head attention, seg-pair interleaved with projection ----
            # scoresT[lk, lq] directly (operands swapped): exp is
            # layout-agnostic (scores provably small -> no max pass), expT
            # feeds AV untransposed.  Softmax denominators: DVE adds the two
            # lk chunks of expT, one batched GpSimd partition_all_reduce per
            # head sums over partitions (all partitions get the result), DVE
            # reciprocal + multiply normalize at the AV psum drain.
            # Schedule: the ~0.7us ACT exp of a seg-pair always has a full
            # projection chunk (~7us) between its scoresT and its AV, so the
            # PE stream never waits on ScalarE.
            head_state = {}

            def emit_scores_pair(h, qh, kh, pair):
                """scoresT + exp + chunk-add for segs 2*pair, 2*pair+1."""
                st = head_state[h]
                for seg in (2 * pair, 2 * pair + 1):
                    scT = pa_p.tile([128, 2, L], F32, tag="pa", name="scT")
                    for lkc in range(2):
                        nc.tensor.matmul(
                            scT[:, lkc, :],
                            kh[:, seg * L + lkc * 128: seg * L + (lkc + 1) * 128],
                            qh[:, seg * L:(seg + 1) * L],
                        )
                    e_t = ex_p.tile([128, 2, L], BF, tag="ex")
                    nc.scalar.activation(
                        out=e_t[:],
                        in_=scT[:],
                        func=mybir.ActivationFunctionType.Exp,
                        scale=SCALE,
                    )
                    st["e"][seg] = e_t
                    nc.vector.tensor_add(
                        st["es"][:, seg, :], e_t[:, 0, :], e_t[:, 1, :])

            def emit_av_pair(h, pair):
                """AV for segs 2*pair, 2*pair+1 into one psum bank."""
                st = head_state[h]
                avs = pa_p.tile([128, 2, L], F32, tag="pav", bufs=2,
                                name="avs")
                st["av"][pair] = avs
                for i, seg in enumerate((2 * pair, 2 * pair + 1)):
                    e_t = st["e"][seg]
                    for lkc in range(2):
                        nc.tensor.matmul(
                            avs[:, i, :],
                            vtok_sb[:, h, seg * 2 + lkc, :],
                            e_t[:, lkc, :],
                            start=(lkc == 0),
                            stop=(lkc == 1),
                        )

            def emit_finalize(h):
                """Batched denominator all-reduce + normalize -> aT_sb."""
                st = head_state[h]
                den = st_p.tile([128, SPC, L], F32, tag="den", bufs=2)
                nc.gpsimd.partition_all_reduce(
                    den[:], st["es"][:], 128, bass_isa.ReduceOp.add)
                inv = st_p.tile([128, SPC, L], F32, tag="inv", bufs=2)
                nc.vector.reciprocal_approx_fast(out=inv[:], in_=den[:])
                for pair in range(2):
                    avs = st["av"][pair]
                    for i, seg in enumerate((2 * pair, 2 * pair + 1)):
                        nc.vector.tensor_mul(
                            aT_sb[:, seg, h, :], avs[:, i, :],
                            inv[:, seg, :])
                del head_state[h]

            def start_head(h, qh, kh):
                head_state[h] = {
                    "q": qh, "k": kh,
                    "e": [None] * SPC,
                    "es": ex_p.tile([128, SPC, L], BF, tag="es", bufs=2, name="es"),
                    "av": [None] * 2,
                }

            prev = None
            for h in range(H):
                qh = qk_p.tile([128, TOK], BF, tag="qk")
                kh = qk_p.tile([128, TOK], BF, tag="qk")
                proj_chunk(h, qh)
                if prev is not None:
                    # block X: AV(prev, pair 0), scoresT(prev, pair 1)
                    emit_av_pair(prev, 0)
                    emit_scores_pair(prev, head_state[prev]["q"],
                                     head_state[prev]["k"], 1)
                proj_chunk(16 + h, kh)
                if prev is not None:
                    # block Y: AV(prev, pair 1) + finalize(prev), then
                    # scoresT(h, pair 0)
                    emit_av_pair(prev, 1)
                    emit_finalize(prev)
                start_head(h, qh, kh)
                emit_scores_pair(h, qh, kh, 0)
                prev = h
            # drain last head
            emit_av_pair(prev, 0)
            emit_scores_pair(prev, head_state[prev]["q"],
                             head_state[prev]["k"], 1)
            emit_av_pair(prev, 1)
            emit_finalize(prev)

            # ---- output projection (feature-major: outT[e, tok]) ----
            # Stationary = Wout tile (one per (e-chunk, head), reused for
            # 1024 moving columns -> 256 LDWEIGHTS total).  Wout streams in
            # sixteen 512KB e-chunk blocks (linear DMAs, 2 prefetched).
            for ec in range(DT):
                wo_ec = w_p.tile([128, H, 128], BF, tag="wo", bufs=2,
                                 name="wo_ec")
                nc.sync.dma_start(out=wo_ec[:], in_=wout_d[ec])
                pos = [pp_p.tile([128, 512], F32, tag="pp", name=f"po{sp}")
                       for sp in range(2)]
                for hh in range(H):
                    for sp in range(2):
                        nc.tensor.matmul(
                            pos[sp][:],
                            wo_ec[:, hh, :],
                            aT_sb[:, 2 * sp:2 * sp + 2, hh, :],
                            start=(hh == 0),
                            stop=(hh == H - 1),
                        )
                for sp in range(2):
                    if ec == DT - 1 and sp == 1:
                        # split the last drain so the final store DMA
                        # starts half a tile earlier
                        for qq in range(2):
                            ob = ou_p.tile([128, 256], F32, tag="ou2")
                            nc.vector.tensor_copy(
                                out=ob[:], in_=pos[sp][:, ts(qq, 256)])
                            nc.sync.dma_start(
                                out=out_d[ec * 128:(ec + 1) * 128,
                                          sp * 512 + qq * 256:
                                          sp * 512 + (qq + 1) * 256],
                                in_=ob[:],
                            )
                    else:
                        ob = ou_p.tile([128, 512], F32, tag="ou")
                        nc.vector.tensor_copy(out=ob[:], in_=pos[sp][:])
                        nc.sync.dma_start(
                            out=out_d[ec * 128:(ec + 1) * 128,
                                      ts(sp, 512)],
                            in_=ob[:],
                        )

    nc.compile()
    _dedupe_ldweights(nc)
    return nc


def _dedupe_ldweights(nc):
    """Drop InstLdweights whose weights are already resident in the PE array.

    tile_legalize emits one LDWEIGHTS per matmul; consecutive matmuls that
    share the stationary operand (projection token-halves, out-proj seg
    pairs) reload identical weights, costing ~97ns of PE pipe each.  Walk
    each block's PE stream tracking the loaded-weights key and delete
    reloads.  Only semaphore-free LDWEIGHTS are dropped, so the sync graph
    is untouched; EVENT_SEMAPHORE/DRAIN between pairs don't disturb the
    array, any other PE instruction conservatively invalidates the key.
    """
    from concourse import mybir

    PE = mybir.EngineType.PE
    dropped = 0
    for f in nc.m.functions:
        for blk in f.blocks:
            insts = blk.instructions
            loaded = None
            to_drop = []
            for idx, x in enumerate(insts):
                if getattr(x, "engine", None) != PE:
                    continue
                nm = type(x).__name__
                if nm == "InstLdweights":
                    si = x.sync_info
                    clean = si is None or (not si.on_wait and not si.on_update)
                    key = (str(x.ins[0]), str(x.is_transpose),
                           str(x.perf_mode), str(x.tile_position))
                    if clean and loaded == key:
                        to_drop.append(idx)
                    else:
                        loaded = key
                elif nm == "InstMatmult":
                    continue
                elif nm in ("InstEventSemaphore", "InstDrain"):
                    continue
                else:
                    loaded = None
            for idx in reversed(to_drop):
                del insts[idx]
            blk.instructions = insts
            dropped += len(to_drop)
    return dropped


def get_program():
    global _PROGRAM
    if _PROGRAM is None:
        _PROGRAM = _build_program()
    return _PROGRAM


def make_in_maps(x, Wqkv, b_qkv):
    """Host-side shard + layout prep (bf16 casts, transposes, tiling)."""
    bf16 = ml_dtypes.bfloat16
    x = np.asarray(x, dtype=np.float32)
    Wqkv = np.asarray(Wqkv, dtype=np.float32)
    b_qkv = np.asarray(b_qkv, dtype=np.float32)

    xs = x.reshape(B, NSEG, SEGMENT, D)[:, :, ::DIL, :]     # [2,16,256,2048]
    xs_flat = xs.reshape(PAIRS, L, D)

    # lhsT tiles packed partition-major: wt[c, p, dt*128+j] = WqkvT[dt*128+p,
    # c*128+j] so one chunk is a single linear per-partition DMA.
    wt = np.ascontiguousarray(
        Wqkv.reshape(NCHUNK, 128, DT, 128).transpose(0, 3, 2, 1)
        .reshape(NCHUNK, 128, DT * 128)
    ).astype(bf16)                                          # [48,128,2048]
    bqt = np.ascontiguousarray(b_qkv.reshape(NCHUNK, 128).T)  # [128,48] f32

    in_maps = []
    for i in range(N_CORES):
        tok = xs_flat[SPC * i:SPC * (i + 1)].reshape(TOK, D)
        xst = np.ascontiguousarray(
            tok.T.reshape(DT, 128, TOK).transpose(1, 0, 2)).astype(bf16)
        in_maps.append({"xst": xst, "wqkv_t": wt, "bq_t": bqt})
    return in_maps


def make_wout_tiled(Wout):
    Wout = np.asarray(Wout, dtype=np.float32)
    # wout_t[ec, p, h*128+j] = Wout[ec*128+j, h*128+p]: per e-chunk block
    # of per-head lhsT tiles, one linear 512KB DMA each.
    return np.ascontiguousarray(
        Wout.reshape(DT, 128, H, 128).transpose(0, 3, 2, 1)
        .reshape(DT, 128, H * 128)).astype(ml_dtypes.bfloat16)


def kernel(x, Wqkv, b_qkv, Wout, b_out):
    from concourse import bass_utils

    nc = get_program()
    in_maps = make_in_maps(x, Wqkv, b_qkv)
    wot = make_wout_tiled(Wout)
    for m in in_maps:
        m["wout_t"] = wot

    res = bass_utils.run_bass_kernel_spmd(
        nc, in_maps, core_ids=list(range(N_CORES)))
    # out is feature-major [D, TOK] per core -> transpose back to [TOK, D]
    outs = [np.ascontiguousarray(res.results[i]["out"].T)
            for i in range(N_CORES)]
    full = np.concatenate(outs, axis=0) + np.asarray(b_out, dtype=np.float32)
    return np.ascontiguousarray(full.reshape(B, NSEG * L, D), dtype=np.float32)


# revision 11
# speedup vs baseline: 1.0492x; 1.0140x over previous
"""Dilated segment attention on 8 TRN2 NeuronCores (Bass/Tile).

Problem (hardcoded from spec):
  x [2, 8192, 2048] f32, Wqkv [6144, 2048], b_qkv [6144], Wout [2048, 2048],
  b_out [2048].  segment=512, dilation=2 -> 16 segments of L=256 dilated
  tokens per segment per batch; per-segment 16-head attention (hd=128);
  fused qkv and out projections.  Output [2, 4096, 2048] f32.

Sharding: the 32 (batch, segment) instances are independent -> 4 per core.
Host pre-gathers the dilated tokens, pre-transposes/pre-tiles operands and
casts to bf16 (compute precision; measured end-to-end rel err ~5e-3).

The kernel is PE-bound (93%+ occupancy): 1.11M matmul columns/core at
2.4GHz is ~464us.  Beyond the baseline, this version
  - computes softmax denominators with a per-head batched GpSimd
    partition_all_reduce instead of ones-matmuls (-32k PE columns),
  - interleaves attention seg-pairs between the q- and k-projection
    chunks so the ACT exp latency always hides under ~7us of projection,
  - emits the output projection feature-major (outT[e, tok]) so the
    stationary operand is a Wout tile reused for 1024 moving columns
    (half the LDWEIGHTS of the token-major form); host transposes back,
  - streams xst by token-quarters and runs the first v-chunk
    quarter-major so the PE starts ~4us earlier during the cold 4MB
    xst delivery.

Per-core dataflow (all matmuls K=128, bf16):
  qkv proj   : feature-major  qkvT[e, tok] = W-tile.T @ xsT-tile  (accum 16)
  scores     : scoresT[lk, lq] = kT.T @ qT  (per seg, head)
  softmax    : exp on ScalarE (scale=1/sqrt(hd); scores provably in
               [-6, 6] so no max subtraction); denominators via DVE
               chunk-add + GpSimd partition_all_reduce; normalize on DVE
               at the AV psum drain.
  AV         : outT[hd, lq] = v[lk, hd].T @ expT[lk, lq]
  out proj   : outT[e, tok] = WoutT-tile.T @ aT-tile  (accum 16 heads)
b_out is applied on the host (purely linear post-op); b_qkv is applied
on-chip (ScalarE bias) since it feeds the softmax nonlinearity.
"""

import numpy as np
import ml_dtypes

B = 2
S = 8192
D = 2048
H = 16
HD = 128
SEGMENT = 512
DIL = 2
NSEG = S // SEGMENT          # 16
L = SEGMENT // DIL           # 256 dilated tokens per segment
N_CORES = 8
PAIRS = B * NSEG             # 32 independent (b, n) instances
SPC = PAIRS // N_CORES       # 4 segments per core
TOK = SPC * L                # 1024 tokens per core
DT = D // 128                # 16 contraction tiles
NCHUNK = 3 * D // 128        # 48 qkv feature chunks (16 q, 16 k, 16 v)
SCALE = 1.0 / float(np.sqrt(HD))

_PROGRAM = None


def _build_program():
    import concourse.bass as bass
    import concourse.bacc as bacc
    import concourse.tile as tile
    from concourse import mybir
    from concourse import bass_isa

    BF = mybir.dt.bfloat16
    F32 = mybir.dt.float32
    ts = bass.ts

    nc = bacc.Bacc("TRN2", target_bir_lowering=False, debug=False,
                   num_devices=N_CORES)

    xst_d = nc.dram_tensor("xst", [4, 128, DT, 256], BF, kind="ExternalInput")
    wqkv_d = nc.dram_tensor("wqkv_t", [NCHUNK, 128, DT * 128], BF,
                            kind="ExternalInput")
    wout_d = nc.dram_tensor("wout_t", [DT, 128, H * 128], BF,
                            kind="ExternalInput")
    bq_d = nc.dram_tensor("bq_t", [128, NCHUNK], F32, kind="ExternalInput")
    out_d = nc.dram_tensor("out", [D, TOK], F32, kind="ExternalOutput")

    with tile.TileContext(nc) as tc:
        with (
            tc.tile_pool(name="const", bufs=1) as const_p,
            tc.tile_pool(name="big", bufs=1) as big_p,
            tc.tile_pool(name="wq", bufs=8) as w_p,
            tc.tile_pool(name="qk", bufs=4) as qk_p,
            tc.tile_pool(name="vt", bufs=2) as vt_p,
            tc.tile_pool(name="ex", bufs=4) as ex_p,
            tc.tile_pool(name="st", bufs=2) as st_p,
            tc.tile_pool(name="ou", bufs=2) as ou_p,
            tc.tile_pool(name="pp", bufs=4, space="PSUM") as pp_p,
            tc.tile_pool(name="pa", bufs=2, space="PSUM") as pa_p,
        ):
            bq_sb = const_p.tile([128, NCHUNK], F32)
            nc.sync.dma_start(out=bq_sb[:], in_=bq_d[:])

            # Cold-start ordering: the PE's first work (v chunk 0, index
            # 32) needs the first quarter of its W chunk and the first
            # token-quarter of xst.  Both live in token-quarter-major
            # layouts so every piece is one fully-linear DMA, and the
            # first matmul can start after ~1.1MB instead of ~4.5MB.
            first_w = w_p.tile([128, DT * 128], BF, tag="w", name="first_w")
            nc.sync.dma_start(out=first_w[:, 0:512], in_=wqkv_d[32][:, 0:512])
            xst_sb = big_p.tile([128, 4, DT, 256], BF)
            nc.sync.dma_start(out=xst_sb[:, 0], in_=xst_d[0])
            for kk in range(1, 4):
                nc.sync.dma_start(out=first_w[:, ts(kk, 512)],
                                  in_=wqkv_d[32][:, ts(kk, 512)])
            for q in range(1, 4):
                nc.sync.dma_start(out=xst_sb[:, q], in_=xst_d[q])
            vtok_sb = big_p.tile([128, H, SPC * 2, 128], BF)
            aT_sb = big_p.tile([128, SPC, H, L], BF)

            def proj_chunk(c, out_tile, wck=None, quarter_major=False):
                """qkvT chunk c: out_tile[128, TOK] bf16 = W-chunk.T @ xsT + b."""
                if wck is None:
                    wck = w_p.tile([128, DT * 128], BF, tag="w")
                    nc.sync.dma_start(out=wck[:], in_=wqkv_d[c])
                if quarter_major:
                    # first chunk while xst streams in: consume one
                    # token-quarter at a time so matmuls start on quarter 0
                    for q in range(4):
                        psq = pp_p.tile([128, 512], F32, tag="pp", name="psq")
                        for dt in range(DT):
                            nc.tensor.matmul(
                                psq[:, 0:256],
                                wck[:, ts(dt, 128)],
                                xst_sb[:, q, dt, :],
                                start=(dt == 0),
                                stop=(dt == DT - 1),
                            )
                        nc.scalar.activation(
                            out=out_tile[:, ts(q, 256)],
                            in_=psq[:, 0:256],
                            func=mybir.ActivationFunctionType.Identity,
                            bias=bq_sb[:, c:c + 1],
                            scale=1.0,
                        )
                    return
                pss = [pp_p.tile([128, 512], F32, tag="pp", name=f"ps{half}")
                       for half in range(2)]
                for dt in range(DT):
                    for half in range(2):
                        nc.tensor.matmul(
                            pss[half][:],
                            wck[:, ts(dt, 128)],
                            xst_sb[:, 2 * half:2 * half + 2, dt, :],
                            start=(dt == 0),
                            stop=(dt == DT - 1),
                        )
                for half in range(2):
                    nc.scalar.activation(
                        out=out_tile[:, ts(half, 512)],
                        in_=pss[half][:],
                        func=mybir.ActivationFunctionType.Identity,
                        bias=bq_sb[:, c:c + 1],
                        scale=1.0,
                    )

            # ---- v projection (feature-major) + transpose to token-major ----
            # One transposing DMA per head (xbar transpose, ~261GB/s): row
            # tok = tc*128+p of vt.T lands at vtok[p, tc, :], exactly the AV
            # stationary layout.  Emitted one chunk behind the projection so
            # the DMA never waits on the ScalarE psum->sbuf drain.
            def v_transposes(h, vt_tile):
                nc.sync.dma_start(out=vtok_sb[:, h, :, :], in_=vt_tile[:],
                                  transpose=True)

            prev_v = None
            for h in range(H):
                vt_tile = vt_p.tile([128, TOK], BF, tag="vt")
                proj_chunk(32 + h, vt_tile,
                           wck=first_w if h == 0 else None,
                           quarter_major=(h == 0))
                if prev_v is not None:
                    v_transposes(h - 1, prev_v)
                prev_v = vt_tile
            v_transposes(H - 1, prev_v)

            # ---- per-head attention, seg-pair interleaved with projection ----
            # scoresT[lk, lq] directly (operands swapped): exp is
            # layout-agnostic (scores provably small -> no max pass), expT
            # feeds AV untransposed.  Softmax denominators: DVE adds the two
            # lk chunks of expT, one batched GpSimd partition_all_reduce per
            # head sums over partitions (all partitions get the result), DVE
            # reciprocal + multiply normalize at the AV psum drain.
            # Schedule: the ~0.7us ACT exp of a seg-pair always has a full
            # projection chunk (~7us) between its scoresT and its AV, so the
            # PE stream never waits on ScalarE.
            head_state = {}

            def emit_scores_pair(h, qh, kh, pair):
                """scoresT + exp + chunk-add for segs 2*pair, 2*pair+1."""
                st = head_state[h]
                for seg in (2 * pair, 2 * pair + 1):
                    scT = pa_p.tile([128, 2, L], F32, tag="pa", name="scT")
                    for lkc in range(2):
                        nc.tensor.matmul(
                            scT[:, lkc, :],
                            kh[:, seg * L + lkc * 128: seg * L + (lkc + 1) * 128],
                            qh[:, seg * L:(seg + 1) * L],
                        )
                    e_t = ex_p.tile([128, 2, L], BF, tag="ex")
                    nc.scalar.activation(
                        out=e_t[:],
                        in_=scT[:],
                        func=mybir.ActivationFunctionType.Exp,
                        scale=SCALE,
                    )
                    st["e"][seg] = e_t
                    nc.vector.tensor_add(
                        st["es"][:, seg, :], e_t[:, 0, :], e_t[:, 1, :])

            def emit_av_pair(h, pair):
                """AV for segs 2*pair, 2*pair+1 into one psum bank."""
                st = head_state[h]
                avs = pa_p.tile([128, 2, L], F32, tag="pav", bufs=2,
                                name="avs")
                st["av"][pair] = avs
                for i, seg in enumerate((2 * pair, 2 * pair + 1)):
                    e_t = st["e"][seg]
                    for lkc in range(2):
                        nc.tensor.matmul(
                            avs[:, i, :],
                            vtok_sb[:, h, seg * 2 + lkc, :],
                            e_t[:, lkc, :],
                            start=(lkc == 0),
                            stop=(lkc == 1),
                        )

            def emit_finalize(h):
                """Batched denominator all-reduce + normalize -> aT_sb."""
                st = head_state[h]
                den = st_p.tile([128, SPC, L], F32, tag="den", bufs=2)
                nc.gpsimd.partition_all_reduce(
                    den[:], st["es"][:], 128, bass_isa.ReduceOp.add)
                inv = st_p.tile([128, SPC, L], F32, tag="inv", bufs=2)
                nc.vector.reciprocal_approx_fast(out=inv[:], in_=den[:])
                for pair in range(2):
                    avs = st["av"][pair]
                    for i, seg in enumerate((2 * pair, 2 * pair + 1)):
                        nc.vector.tensor_mul(
                            aT_sb[:, seg, h, :], avs[:, i, :],
                            inv[:, seg, :])
                del head_state[h]

            def start_head(h, qh, kh):
                head_state[h] = {
                    "q": qh, "k": kh,
                    "e": [None] * SPC,
                    "es": ex_p.tile([128, SPC, L], BF, tag="es", bufs=2, name="es"),
                    "av": [None] * 2,
                }

            # Prefetch the first two Wout e-chunk blocks now: their
            # dma_starts land early in the Sync stream, so the transfers
            # run during the attention phase instead of stalling the
            # out-projection start by ~3us.
            wo_pre = []
            for ec in range(2):
                wo_ec = w_p.tile([128, H, 128], BF, tag="wo", bufs=2,
                                 name="wo_ec")
                nc.sync.dma_start(out=wo_ec[:], in_=wout_d[ec])
                wo_pre.append(wo_ec)

            prev = None
            for h in range(H):
                qh = qk_p.tile([128, TOK], BF, tag="qk")
                kh = qk_p.tile([128, TOK], BF, tag="qk")
                proj_chunk(h, qh)
                if prev is not None:
                    # block X: AV(prev, pair 0), scoresT(prev, pair 1)
                    emit_av_pair(prev, 0)
                    emit_scores_pair(prev, head_state[prev]["q"],
                                     head_state[prev]["k"], 1)
                proj_chunk(16 + h, kh)
                if prev is not None:
                    # block Y: AV(prev, pair 1) + finalize(prev), then
                    # scoresT(h, pair 0)
                    emit_av_pair(prev, 1)
                    emit_finalize(prev)
                start_head(h, qh, kh)
                emit_scores_pair(h, qh, kh, 0)
                prev = h
            # drain last head
            emit_av_pair(prev, 0)
            emit_scores_pair(prev, head_state[prev]["q"],
                             head_state[prev]["k"], 1)
            emit_av_pair(prev, 1)
            emit_finalize(prev)

            # ---- output projection (feature-major: outT[e, tok]) ----
            # Stationary = Wout tile (one per (e-chunk, head), reused for
            # 1024 moving columns -> 256 LDWEIGHTS total).  Wout streams in
            # sixteen 512KB e-chunk blocks (linear DMAs, 2 prefetched).
            for ec in range(DT):
                if ec < 2:
                    wo_ec = wo_pre[ec]
                else:
                    wo_ec = w_p.tile([128, H, 128], BF, tag="wo", bufs=2,
                                     name="wo_ec")
                    nc.sync.dma_start(out=wo_ec[:], in_=wout_d[ec])
                pos = [pp_p.tile([128, 512], F32, tag="pp", name=f"po{sp}")
                       for sp in range(2)]
                for hh in range(H):
                    for sp in range(2):
                        nc.tensor.matmul(
                            pos[sp][:],
                            wo_ec[:, hh, :],
                            aT_sb[:, 2 * sp:2 * sp + 2, hh, :],
                            start=(hh == 0),
                            stop=(hh == H - 1),
                        )
                for sp in range(2):
                    if ec == DT - 1 and sp == 1:
                        # split the last drain so the final store DMA
                        # starts half a tile earlier
                        for qq in range(2):
                            ob = ou_p.tile([128, 256], F32, tag="ou2")
                            nc.vector.tensor_copy(
                                out=ob[:], in_=pos[sp][:, ts(qq, 256)])
                            nc.sync.dma_start(
                                out=out_d[ec * 128:(ec + 1) * 128,
                                          sp * 512 + qq * 256:
                                          sp * 512 + (qq + 1) * 256],
                                in_=ob[:],
                            )
                    else:
                        ob = ou_p.tile([128, 512], F32, tag="ou")
                        nc.vector.tensor_copy(out=ob[:], in_=pos[sp][:])
                        nc.sync.dma_start(
                            out=out_d[ec * 128:(ec + 1) * 128,
                                      ts(sp, 512)],
                            in_=ob[:],
                        )

    nc.compile()
    _dedupe_ldweights(nc)
    return nc


def _dedupe_ldweights(nc):
    """Drop InstLdweights whose weights are already resident in the PE array.

    tile_legalize emits one LDWEIGHTS per matmul; consecutive matmuls that
    share the stationary operand (projection token-halves, out-proj seg
    pairs) reload identical weights, costing ~97ns of PE pipe each.  Walk
    each block's PE stream tracking the loaded-weights key and delete
    reloads.  Only semaphore-free LDWEIGHTS are dropped, so the sync graph
    is untouched; EVENT_SEMAPHORE/DRAIN between pairs don't disturb the
    array, any other PE instruction conservatively invalidates the key.
    """
    from concourse import mybir

    PE = mybir.EngineType.PE
    dropped = 0
    for f in nc.m.functions:
        for blk in f.blocks:
            insts = blk.instructions
            loaded = None
            to_drop = []
            for idx, x in enumerate(insts):
                if getattr(x, "engine", None) != PE:
                    continue
                nm = type(x).__name__
                if nm == "InstLdweights":
                    si = x.sync_info
                    clean = si is None or (not si.on_wait and not si.on_update)
                    key = (str(x.ins[0]), str(x.is_transpose),
                           str(x.perf_mode), str(x.tile_position))
                    if clean and loaded == key:
                        to_drop.append(idx)
                    else:
                        loaded = key
                elif nm == "InstMatmult":
                    continue
                elif nm in ("InstEventSemaphore", "InstDrain"):
                    continue
                else:
                    loaded = None
            for idx in reversed(to_drop):
                del insts[idx]
            blk.instructions = insts
            dropped += len(to_drop)
    return dropped


def get_program():
    global _PROGRAM
    if _PROGRAM is None:
        _PROGRAM = _build_program()
    return _PROGRAM


def make_in_maps(x, Wqkv, b_qkv):
    """Host-side shard + layout prep (bf16 casts, transposes, tiling)."""
    bf16 = ml_dtypes.bfloat16
    x = np.asarray(x, dtype=np.float32)
    Wqkv = np.asarray(Wqkv, dtype=np.float32)
    b_qkv = np.asarray(b_qkv, dtype=np.float32)

    xs = x.reshape(B, NSEG, SEGMENT, D)[:, :, ::DIL, :]     # [2,16,256,2048]
    xs_flat = xs.reshape(PAIRS, L, D)

    # lhsT tiles packed partition-major: wt[c, p, dt*128+j] = WqkvT[dt*128+p,
    # c*128+j] so one chunk is a single linear per-partition DMA.
    wt = np.ascontiguousarray(
        Wqkv.reshape(NCHUNK, 128, DT, 128).transpose(0, 3, 2, 1)
        .reshape(NCHUNK, 128, DT * 128)
    ).astype(bf16)                                          # [48,128,2048]
    bqt = np.ascontiguousarray(b_qkv.reshape(NCHUNK, 128).T)  # [128,48] f32

    in_maps = []
    for i in range(N_CORES):
        tok = xs_flat[SPC * i:SPC * (i + 1)].reshape(TOK, D)
        # token-quarter-major: xst[q, p, dt, j] = xsT[dt*128+p, q*256+j]
        # so each quarter is one fully-linear DMA.
        xst = np.ascontiguousarray(
            tok.T.reshape(DT, 128, 4, 256).transpose(2, 1, 0, 3)).astype(bf16)
        in_maps.append({"xst": xst, "wqkv_t": wt, "bq_t": bqt})
    return in_maps


def make_wout_tiled(Wout):
    Wout = np.asarray(Wout, dtype=np.float32)
    # wout_t[ec, p, h*128+j] = Wout[ec*128+j, h*128+p]: per e-chunk block
    # of per-head lhsT tiles, one linear 512KB DMA each.
    return np.ascontiguousarray(
        Wout.reshape(DT, 128, H, 128).transpose(0, 3, 2, 1)
        .reshape(DT, 128, H * 128)).astype(ml_dtypes.bfloat16)


def kernel(x, Wqkv, b_qkv, Wout, b_out):
    from concourse import bass_utils

    nc = get_program()
    in_maps = make_in_maps(x, Wqkv, b_qkv)
    wot = make_wout_tiled(Wout)
    for m in in_maps:
        m["wout_t"] = wot

    res = bass_utils.run_bass_kernel_spmd(
        nc, in_maps, core_ids=list(range(N_CORES)))
    # out is feature-major [D, TOK] per core -> transpose back to [TOK, D]
    outs = [np.ascontiguousarray(res.results[i]["out"].T)
            for i in range(N_CORES)]
    full = np.concatenate(outs, axis=0) + np.asarray(b_out, dtype=np.float32)
    return np.ascontiguousarray(full.reshape(B, NSEG * L, D), dtype=np.float32)
